# revision 4
# baseline (speedup 1.0000x reference)
"""Trainium2 Bass kernel for DigitConvolutionalModel (conv3x3 -> FC512 -> FC10).

Math: the 3x3 valid conv is linear, so  y_flat = x @ C  with C [784, 676]
holding conv_w values in a banded structure.  Then
    logits = relu(x @ (C @ W1) + b1) @ W2 + b2
The fold W1_eff = C @ W1 is computed on device (banded matmul over only
the nonzero blocks), then the big [2048, 784] @ [784, 512] matmul per
core, relu, and the [*, 512] @ [512, 10] head.  Data-parallel across 8
cores on the batch dim.

Host-side work is layout/dtype only: shard + transpose x, cast to bf16,
arrange C^T / W1 / b1 / W2 / b2 into SBUF-friendly layouts.
"""

import numpy as np
import ml_dtypes

B = 16384
IMG = 28
K = 3
OUT = IMG - K + 1  # 26
M26 = OUT * OUT  # 676
Q = IMG * IMG  # 784
HID = 512
NCLS = 10

NCORES = 8
BL = B // NCORES  # 2048 rows per core
QT = 112  # q-tile height (partition dim), 7 tiles
NQT = Q // QT  # 7
SB = 512  # batch superblock (matmul N)
NSB = BL // SB  # 4
NHT = HID // 128  # 4
NMC = (M26 + 127) // 128  # 6 m-chunks
NWARM = 20  # dummy matmuls to warm the PE/HAM during the DMA prologue

TRACE = False  # set by test harness to capture an NTFF profile
_CACHE = {}

_BF16 = ml_dtypes.bfloat16


def _band_blocks():
    """Static nonzero block pattern of C^T [676, 784] against (mc, qt) tiling.

    Returns per q-tile the list of m-chunk indices whose [128, QT] block of
    C^T contains structural nonzeros.
    """
    Cs = np.zeros((Q, M26), dtype=bool)
    ii, jj = np.meshgrid(np.arange(OUT), np.arange(OUT), indexing="ij")
    m = (OUT * ii + jj).ravel()
    for di in range(K):
        for dj in range(K):
            q = ((ii + di) * IMG + (jj + dj)).ravel()
            Cs[q, m] = True
    CT = Cs.T  # [676, 784]
    blocks = []
    for t in range(NQT):
        mcs = []
        for mc in range(NMC):
            rows = min(128, M26 - 128 * mc)
            if CT[128 * mc : 128 * mc + rows, QT * t : QT * (t + 1)].any():
                mcs.append(mc)
        blocks.append(mcs)
    return blocks


_BLOCKS = _band_blocks()
# flat list of (t, mc) pairs; the packed cmat input carries one [128, QT]
# block per pair, in this order
_PAIRS = [(t, mc) for t in range(NQT) for mc in _BLOCKS[t]]


def _build():
    import concourse.bacc as bacc
    import concourse.mybir as mybir
    import concourse.tile as tile

    f32 = mybir.dt.float32
    bf16 = mybir.dt.bfloat16
    AF = mybir.ActivationFunctionType

    nc = bacc.Bacc("TRN2", target_bir_lowering=False, debug=False)

    xt_d = nc.dram_tensor("xt", [Q, BL], bf16, kind="ExternalInput")
    cm_d = nc.dram_tensor("cmb", [len(_PAIRS), 128, QT], bf16, kind="ExternalInput")
    w1_d = nc.dram_tensor("w1", [M26, HID], bf16, kind="ExternalInput")
    b1_d = nc.dram_tensor("b1l", [128, NHT], f32, kind="ExternalInput")
    w2_d = nc.dram_tensor("w2l", [128, NHT * NCLS], bf16, kind="ExternalInput")
    b2_d = nc.dram_tensor("b2l", [NCLS, 1], f32, kind="ExternalInput")
    out_d = nc.dram_tensor("out", [NCLS, BL], f32, kind="ExternalOutput")

    with tile.TileContext(nc) as tc:
        with (
            tc.tile_pool(name="weights", bufs=1) as wp,
            tc.tile_pool(name="xin", bufs=2) as xp,
            tc.tile_pool(name="hid", bufs=2) as hp,
            tc.tile_pool(name="lgts", bufs=2) as lp,
            tc.tile_pool(name="psF", bufs=1, space="PSUM") as psF,
            tc.tile_pool(name="ps1", bufs=5, space="PSUM") as ps1p,
            tc.tile_pool(name="ps2", bufs=2, space="PSUM") as ps2p,
        ):
            # ---- PE warmup: dependency-free matmuls on scratch data ----
            # These issue immediately (no DMA deps), keeping the PE busy
            # through the weight-DMA prologue so HAM reaches K=8/8 before
            # real work, instead of the whole first half running at 1.2 GHz.
            scratch = wp.tile([128, HID], bf16, tag="scratch")
            nc.vector.memset(scratch[:], 0.0)
            warm = psF.tile([128, HID], f32, tag="ps")
            for i in range(NWARM):
                nc.tensor.matmul(
                    warm[:],
                    lhsT=scratch[:, :128],
                    rhs=scratch[:],
                    start=True,
                    stop=True,
                )

            # ---- load weights ----
            cms = {}
            for p, (t, mc) in enumerate(_PAIRS):
                cm = wp.tile([128, QT], bf16, tag=f"cm{p}")
                nc.sync.dma_start(out=cm[:], in_=cm_d[p, :, :])
                cms[(t, mc)] = cm
            w1s = []
            for i in range(NMC):
                rows = min(128, M26 - 128 * i)
                w1 = wp.tile([128, HID], bf16, tag=f"w1_{i}")
                nc.sync.dma_start(out=w1[:rows], in_=w1_d[128 * i : 128 * i + rows, :])
                w1s.append(w1)
            b1 = wp.tile([128, NHT], f32, tag="b1")
            nc.sync.dma_start(out=b1[:], in_=b1_d[:, :])
            w2 = wp.tile([128, NHT * NCLS], bf16, tag="w2")
            nc.sync.dma_start(out=w2[:], in_=w2_d[:, :])
            b2 = wp.tile([NCLS, 1], f32, tag="b2")
            nc.sync.dma_start(out=b2[:], in_=b2_d[:, :])

            # ---- fold: W1_eff[q, h] = sum_m C^T[m, q] * W1[m, h] ----
            w1eff = []
            for t in range(NQT):
                ps = psF.tile([QT, HID], f32, tag="ps")
                mcs = _BLOCKS[t]
                for j, mc in enumerate(mcs):
                    rows = min(128, M26 - 128 * mc)
                    nc.tensor.matmul(
                        ps[:],
                        lhsT=cms[(t, mc)][:rows, :],
                        rhs=w1s[mc][:rows, :],
                        start=(j == 0),
                        stop=(j == len(mcs) - 1),
                    )
                we = wp.tile([QT, HID], bf16, tag=f"we{t}")
                nc.vector.tensor_copy(we[:], ps[:])
                w1eff.append(we)

            # ---- main loop over batch superblocks ----
            for s in range(NSB):
                xts = []
                for t in range(NQT):
                    xt = xp.tile([QT, SB], bf16, tag=f"xt{t}")
                    nc.scalar.dma_start(
                        out=xt[:], in_=xt_d[QT * t : QT * (t + 1), SB * s : SB * (s + 1)]
                    )
                    xts.append(xt)
                hs = []
                for ht in range(NHT):
                    ps1 = ps1p.tile([128, SB], f32)
                    for t in range(NQT):
                        nc.tensor.matmul(
                            ps1[:],
                            lhsT=w1eff[t][:, 128 * ht : 128 * (ht + 1)],
                            rhs=xts[t][:],
                            start=(t == 0),
                            stop=(t == NQT - 1),
                        )
                    h = hp.tile([128, SB], bf16, tag=f"h{ht}")
                    nc.scalar.activation(
                        h[:], ps1[:], AF.Relu, bias=b1[:, ht : ht + 1], scale=1.0
                    )
                    hs.append(h)
                ps2 = ps2p.tile([NCLS, SB], f32)
                for ht in range(NHT):
                    nc.tensor.matmul(
                        ps2[:],
                        lhsT=w2[:, NCLS * ht : NCLS * (ht + 1)],
                        rhs=hs[ht][:],
                        start=(ht == 0),
                        stop=(ht == NHT - 1),
                    )
                lg = lp.tile([NCLS, SB], f32, tag="lg")
                nc.scalar.activation(
                    lg[:], ps2[:], AF.Identity, bias=b2[:, 0:1], scale=1.0
                )
                nc.sync.dma_start(out=out_d[:, SB * s : SB * (s + 1)], in_=lg[:])

    nc.compile()
    return nc


def _get_nc():
    if "nc" not in _CACHE:
        _CACHE["nc"] = _build()
    return _CACHE["nc"]


def kernel(x, conv_w, W1, b1, W2, b2):
    from concourse.bass_utils import run_bass_kernel_spmd

    nc = _get_nc()

    # C [784, 676]: y_flat = x @ C  (banded placement of conv_w values)
    C = np.zeros((Q, M26), dtype=np.float32)
    ii, jj = np.meshgrid(np.arange(OUT), np.arange(OUT), indexing="ij")
    m = (OUT * ii + jj).ravel()
    cw = np.asarray(conv_w, dtype=np.float32)
    for di in range(K):
        for dj in range(K):
            q = ((ii + di) * IMG + (jj + dj)).ravel()
            C[q, m] = cw[di, dj]
    CT = C.T  # [676, 784]
    cmb = np.zeros((len(_PAIRS), 128, QT), dtype=np.float32)
    for p, (t, mc) in enumerate(_PAIRS):
        rows = min(128, M26 - 128 * mc)
        cmb[p, :rows, :] = CT[128 * mc : 128 * mc + rows, QT * t : QT * (t + 1)]
    cmb = cmb.astype(_BF16)

    w1l = np.asarray(W1, np.float32).astype(_BF16)  # [676, 512]
    b1l = np.ascontiguousarray(
        np.asarray(b1, np.float32).reshape(NHT, 128).T
    )  # [128, 4]
    w2l = np.ascontiguousarray(
        np.asarray(W2, np.float32)
        .reshape(NHT, 128, NCLS)
        .transpose(1, 0, 2)
        .reshape(128, NHT * NCLS)
    ).astype(_BF16)
    b2l = np.asarray(b2, np.float32).reshape(NCLS, 1)

    xf = np.asarray(x, np.float32)
    in_maps = []
    for c in range(NCORES):
        xt = np.ascontiguousarray(xf[c * BL : (c + 1) * BL].T).astype(_BF16)
        in_maps.append(
            {
                "xt": xt,
                "cmb": cmb,
                "w1": w1l,
                "b1l": b1l,
                "w2l": w2l,
                "b2l": b2l,
            }
        )

    kwargs = {}
    if TRACE:
        import profhook  # noqa: F401  (installs the NTFF hook shim)
        import tempfile

        kwargs = {"trace": True, "tmpdir": tempfile.mkdtemp(prefix="ntff_")}
    res = run_bass_kernel_spmd(nc, in_maps, core_ids=list(range(NCORES)), **kwargs)
    if TRACE:
        _CACHE["last_results"] = res

    out = np.concatenate(
        [np.ascontiguousarray(res.results[c]["out"].T) for c in range(NCORES)], axis=0
    ).astype(np.float32)
    return out
